# revision 9
# baseline (speedup 1.0000x reference)
"""Trainium2 Bass kernel for nn_CustomLoss_17875653886357.

Contrastive-style loss vs. the last row (anchor) of the batch:
    lab  = (labels != labels[-1])                        [N]
    dist = ||coords - coords[-1]||^2                     [N]
    loss = sum((1-lab)*dist + lab*max(0, MARGIN-dist))   scalar

Sharding: data-parallel over N across 8 NeuronCores (4096 rows each).
The anchor row (3 floats + 1 int) is baked into the compiled kernel as
immediates; each core produces a scalar partial sum; host adds the 8
partials (the gather/unshard step).

All per-core inputs ride in ONE DMA: a [128, 226] f32 "blob" whose
partition rows pack coords (96) | anchor broadcast (96) | labels as i32
bits (32) | 1.0 | -1.0.  904B/descriptor clears the <512B DMA
latency penalty, there is a single DGE launch + semaphore set, and no
on-device memsets are needed (the matmul constants come in with the
blob) - so nothing issues before the DMA.

Raw Bacc (no Tile framework): the kernel is a straight pipeline, so
hand-placed semaphores avoid Tile's entry branches and tail
drain+barrier+sem-clear sequence.
"""

from contextlib import ExitStack

import numpy as np

import concourse.mybir as mybir
from concourse import bacc
from concourse.bass_utils import run_bass_kernel_spmd

N, D = 32768, 3
NCORES = 8
NS = N // NCORES  # rows per core = 4096
P = 128  # SBUF partitions
M = NS // P  # rows per partition = 32
MARGIN = 500.0

F32 = mybir.dt.float32
I32 = mybir.dt.int32
Alu = mybir.AluOpType

VARIANT = "act"  # "dve" | "act"

CW = M * D  # coords block width = 96
if VARIANT == "dve":
    BW = 2 * CW + M + 2  # coords | anchor bcast | labels | 1.0 | -1.0
else:
    BW = CW + M + 4  # coords | labels | -ax | -ay | -az | 1.0


def _build(anchor_pt, anchor_lab):
    """Build the per-core Bacc program. Anchor values are compile-time
    immediates (the kernel is compiled per call, after inputs are known)."""
    if VARIANT == "act":
        return _build_act(anchor_lab)
    al = int(anchor_lab)

    nc = bacc.Bacc(
        "TRN2", target_bir_lowering=False, debug=False, enable_partition_id=False
    )
    bb = nc.cur_bb.bb
    init_names = {i.name for i in bb.instructions}
    blob_d = nc.declare_dram_parameter("blob", [P, BW], F32, isOutput=False)
    out_d = nc.declare_dram_parameter("out", [1, 1], F32, isOutput=True)

    with ExitStack() as ctx:
        BLOB = ctx.enter_context(nc.sbuf_tensor("BLOB", [P, BW], F32))
        DIFF = ctx.enter_context(nc.sbuf_tensor("DIFF", [P, CW], F32))
        SQ = ctx.enter_context(nc.sbuf_tensor("SQ", [P, CW], F32))
        E = ctx.enter_context(nc.sbuf_tensor("E", [P, M], F32))
        DN = ctx.enter_context(nc.sbuf_tensor("DN", [P, M], F32))
        H = ctx.enter_context(nc.sbuf_tensor("H", [P, M], F32))
        B = ctx.enter_context(nc.sbuf_tensor("B", [P, M], F32))
        EM = ctx.enter_context(nc.sbuf_tensor("EM", [P, M], F32))
        LOSS = ctx.enter_context(nc.sbuf_tensor("LOSS", [P, M], F32))
        RS = ctx.enter_context(nc.sbuf_tensor("RS", [P, 1], F32))
        ACC = ctx.enter_context(nc.psum_tensor("ACC", [1, 1], F32))
        in_sem = ctx.enter_context(nc.semaphore("in_sem"))
        v_sem = ctx.enter_context(nc.semaphore("v_sem"))
        pe_sem = ctx.enter_context(nc.semaphore("pe_sem"))
        out_sem = ctx.enter_context(nc.semaphore("out_sem"))

        ap = BLOB[:]
        C = ap[:, 0:CW]
        AB = ap[:, CW : 2 * CW]
        LI = ap[:, 2 * CW : 2 * CW + M].bitcast(I32)
        ONE = ap[:, BW - 2 : BW - 1]

        # One input DMA on sync's HW queue: all 16 phys DMA engines chew
        # the 128 blob descriptors in ~0.4us.
        dma_a = nc.sync.dma_start(BLOB[:], blob_d[:])
        dma_a.then_inc(in_sem, 16)

        # DVE instructions don't interlock with their predecessors' writes
        # (deep pipeline), so every same-engine RAW needs a semaphore hop:
        # each op bumps v_sem, dependent ops wait for the producer's count.
        vs = [0]

        def vop(inst):
            inst.then_inc(v_sem, 1)
            vs[0] += 1
            return vs[0]

        # --- dist path
        nc.vector.wait_ge(in_sem, 16)
        vop(nc.vector.tensor_sub(DIFF[:], C, AB))
        nc.vector.wait_ge(v_sem, vs[0])
        vop(nc.vector.tensor_tensor(SQ[:], DIFF[:], DIFF[:], Alu.mult))
        SQ3 = SQ[:].rearrange("p (m d) -> p m d", d=D)
        nc.vector.wait_ge(v_sem, vs[0])
        vop(
            nc.vector.tensor_reduce(  # DN = -dist
                DN[:], SQ3, axis=mybir.AxisListType.X, op=Alu.add, negate=True
            )
        )
        # H = max(MARGIN - dist, 0) = max(DN + MARGIN, 0)
        nc.vector.wait_ge(v_sem, vs[0])
        h_t = vop(nc.vector.tensor_scalar(H[:], DN[:], MARGIN, 0.0, Alu.add, Alu.max))

        # label path, slotted here: no same-engine dependency, so it
        # overlaps the H->B semaphore hop (labels rode the same blob DMA)
        vop(nc.vector.tensor_scalar(E[:], LI, al, None, Alu.is_equal))

        nc.vector.wait_ge(v_sem, h_t)
        vop(nc.vector.tensor_add(B[:], DN[:], H[:]))  # B = H - dist

        # loss = H - E*B;  RS = per-partition sum(loss)
        nc.vector.wait_ge(v_sem, vs[0])
        vop(nc.vector.tensor_tensor(EM[:], E[:], B[:], Alu.mult))
        nc.vector.wait_ge(v_sem, vs[0])
        rs_t = vop(
            nc.vector.scalar_tensor_tensor(
                LOSS[:], EM[:], -1.0, H[:], Alu.mult, Alu.add, accum_out=RS[:]
            )
        )

        # --- cross-partition reduction on PE: [1,1] = RS.T @ ones
        nc.tensor.wait_ge(v_sem, rs_t)
        nc.tensor.matmul(ACC[:], RS[:], ONE, start=True, stop=True).then_inc(
            pe_sem, 1
        )

        # --- result: PSUM -> SBUF (DMA cannot read PSUM), then DMA out
        OUT = ctx.enter_context(nc.sbuf_tensor("OUT", [1, 1], F32))
        nc.vector.wait_ge(pe_sem, 1)
        out_t = vop(nc.vector.tensor_copy(OUT[:], ACC[:]))
        nc.sync.wait_ge(v_sem, out_t)
        # No completion wait here: the NEFF runtime epilogue drains every
        # engine's DMA queues before signalling completion, which covers
        # this last transfer.
        nc.sync.dma_start(out_d[:], OUT[:], single_packet=True).then_inc(out_sem, 16)

    # Bass.__init__ emits per-engine const-tile memsets plus a full
    # drain + all-engine barrier. This kernel uses none of the const
    # tiles, and the NEFF runtime prologue already syncs all engines, so
    # drop them. Also hoist the HWDGE blob DMA to the very front: it
    # carries a pure access pattern (no registers), so it need not sit
    # behind the ~1.3us per-engine TPB-base loads - issuing first hides
    # that latency behind the DMA flight time.
    strip = {
        i.name
        for i in bb.instructions
        if i.name in init_names
        and type(i).__name__ in ("InstMemset", "InstDrain", "InstEventSemaphore")
    }
    front_names = {dma_a.ins.name}
    kept = [i for i in bb.instructions if i.name not in strip]
    front = [i for i in kept if i.name in front_names]
    rest = [i for i in kept if i.name not in front_names]
    idx = next(k for k, i in enumerate(rest) if i.name.endswith("dummycall")) + 1
    bb.instructions[:] = rest[:idx] + front + rest[idx:]

    nc.compile()
    return nc


def _build_act(anchor_lab):
    """ACT-offload variant: the Activation engine computes the three
    per-coordinate squares (x - a_d)^2 directly - Square with a per-
    partition bias rides in the blob - while DVE runs the label compare
    in parallel, then combines.  Shorter DVE critical chain."""
    al = int(anchor_lab)

    nc = bacc.Bacc(
        "TRN2", target_bir_lowering=False, debug=False, enable_partition_id=False
    )
    bb = nc.cur_bb.bb
    init_names = {i.name for i in bb.instructions}
    blob_d = nc.declare_dram_parameter("blob", [P, BW], F32, isOutput=False)
    out_d = nc.declare_dram_parameter("out", [1, 1], F32, isOutput=True)

    with ExitStack() as ctx:
        BLOB = ctx.enter_context(nc.sbuf_tensor("BLOB", [P, BW], F32))
        SQT = ctx.enter_context(nc.sbuf_tensor("SQT", [P, 3 * M], F32))
        E = ctx.enter_context(nc.sbuf_tensor("E", [P, M], F32))
        T01 = ctx.enter_context(nc.sbuf_tensor("T01", [P, M], F32))
        DN = ctx.enter_context(nc.sbuf_tensor("DN", [P, M], F32))
        H = ctx.enter_context(nc.sbuf_tensor("H", [P, M], F32))
        B = ctx.enter_context(nc.sbuf_tensor("B", [P, M], F32))
        EM = ctx.enter_context(nc.sbuf_tensor("EM", [P, M], F32))
        LOSS = ctx.enter_context(nc.sbuf_tensor("LOSS", [P, M], F32))
        RS = ctx.enter_context(nc.sbuf_tensor("RS", [P, 1], F32))
        ACC = ctx.enter_context(nc.psum_tensor("ACC", [1, 1], F32))
        in_sem = ctx.enter_context(nc.semaphore("in_sem"))
        a_sem = ctx.enter_context(nc.semaphore("a_sem"))
        v_sem = ctx.enter_context(nc.semaphore("v_sem"))
        pe_sem = ctx.enter_context(nc.semaphore("pe_sem"))
        out_sem = ctx.enter_context(nc.semaphore("out_sem"))

        ap = BLOB[:]
        C3 = ap[:, 0:CW].rearrange("p (m d) -> p m d", d=D)
        LI = ap[:, CW : CW + M].bitcast(I32)
        BIAS = [ap[:, CW + M + d : CW + M + d + 1] for d in range(D)]
        ONE = ap[:, BW - 1 : BW]

        dma_a = nc.sync.dma_start(BLOB[:], blob_d[:])
        dma_a.then_inc(in_sem, 16)

        vs = [0]

        def vop(inst):
            inst.then_inc(v_sem, 1)
            vs[0] += 1
            return vs[0]

        # --- squares on ACT: SQT[:, d] = (c_d - a_d)^2, no RAW between
        # the three so no same-engine hops needed
        Sq = mybir.ActivationFunctionType.Square
        nc.scalar.wait_ge(in_sem, 16)
        for d in range(D):
            nc.scalar.activation(
                SQT[:][:, d * M : (d + 1) * M], C3[:, :, d], Sq, bias=BIAS[d]
            ).then_inc(a_sem, 1)

        # --- label compare on DVE, overlapping ACT
        nc.vector.wait_ge(in_sem, 16)
        vop(nc.vector.tensor_scalar(E[:], LI, al, None, Alu.is_equal))

        # --- combine: DN = -dist = -(SQ0 + SQ1) - SQ2
        SQTa = SQT[:]
        nc.vector.wait_ge(a_sem, 2)
        vop(nc.vector.tensor_add(T01[:], SQTa[:, 0:M], SQTa[:, M : 2 * M]))
        nc.vector.wait_ge(a_sem, 3)
        nc.vector.wait_ge(v_sem, vs[0])
        vop(
            nc.vector.scalar_tensor_tensor(
                DN[:], T01[:], -1.0, SQTa[:, 2 * M : 3 * M], Alu.mult, Alu.subtract
            )
        )
        # H = max(MARGIN - dist, 0) = max(DN + MARGIN, 0)
        nc.vector.wait_ge(v_sem, vs[0])
        h_t = vop(nc.vector.tensor_scalar(H[:], DN[:], MARGIN, 0.0, Alu.add, Alu.max))

        nc.vector.wait_ge(v_sem, h_t)
        vop(nc.vector.tensor_add(B[:], DN[:], H[:]))  # B = H - dist
        nc.vector.wait_ge(v_sem, vs[0])
        vop(nc.vector.tensor_tensor(EM[:], E[:], B[:], Alu.mult))
        nc.vector.wait_ge(v_sem, vs[0])
        rs_t = vop(
            nc.vector.scalar_tensor_tensor(
                LOSS[:], EM[:], -1.0, H[:], Alu.mult, Alu.add, accum_out=RS[:]
            )
        )

        nc.tensor.wait_ge(v_sem, rs_t)
        nc.tensor.matmul(ACC[:], RS[:], ONE, start=True, stop=True).then_inc(
            pe_sem, 1
        )

        OUT = ctx.enter_context(nc.sbuf_tensor("OUT", [1, 1], F32))
        nc.vector.wait_ge(pe_sem, 1)
        out_t = vop(nc.vector.tensor_copy(OUT[:], ACC[:]))
        nc.sync.wait_ge(v_sem, out_t)
        nc.sync.dma_start(out_d[:], OUT[:], single_packet=True).then_inc(out_sem, 16)

    strip = {
        i.name
        for i in bb.instructions
        if i.name in init_names
        and type(i).__name__ in ("InstMemset", "InstDrain", "InstEventSemaphore")
    }
    front_names = {dma_a.ins.name}
    kept = [i for i in bb.instructions if i.name not in strip]
    front = [i for i in kept if i.name in front_names]
    rest = [i for i in kept if i.name not in front_names]
    idx = next(k for k, i in enumerate(rest) if i.name.endswith("dummycall")) + 1
    bb.instructions[:] = rest[:idx] + front + rest[idx:]

    nc.compile()
    return nc


_nc_cache = {}


def build_nc_and_inmaps(batched_labels, batched_predicted_coords):
    labels = np.ascontiguousarray(batched_labels)
    coords = np.ascontiguousarray(batched_predicted_coords, dtype=np.float32)
    assert labels.shape == (N,) and coords.shape == (N, D)
    if labels.dtype != np.int32:
        labels = labels.astype(np.int32)

    key = (coords[-1].tobytes(), int(labels[-1]))
    nc = _nc_cache.get(key)
    if nc is None:
        nc = _nc_cache[key] = _build(coords[-1], labels[-1])

    in_maps = []
    if VARIANT == "dve":
        ab_row = np.tile(coords[-1], M)  # [96], anchor broadcast
        for i in range(NCORES):
            sl = slice(i * NS, (i + 1) * NS)
            blob = np.empty((P, BW), np.float32)
            blob[:, 0:CW] = coords[sl].reshape(P, CW)
            blob[:, CW : 2 * CW] = ab_row
            blob[:, 2 * CW : 2 * CW + M] = (
                labels[sl].reshape(P, M).view(np.float32)
            )
            blob[:, BW - 2] = 1.0
            blob[:, BW - 1] = -1.0
            in_maps.append({"blob": blob})
    else:
        for i in range(NCORES):
            sl = slice(i * NS, (i + 1) * NS)
            blob = np.empty((P, BW), np.float32)
            blob[:, 0:CW] = coords[sl].reshape(P, CW)
            blob[:, CW : CW + M] = labels[sl].reshape(P, M).view(np.float32)
            blob[:, CW + M : CW + M + D] = -coords[-1]
            blob[:, BW - 1] = 1.0
            in_maps.append({"blob": blob})
    return nc, in_maps


def kernel(batched_labels, batched_predicted_coords, _trace=False, _results=[None]):
    nc, in_maps = build_nc_and_inmaps(batched_labels, batched_predicted_coords)
    res = run_bass_kernel_spmd(nc, in_maps, core_ids=list(range(NCORES)), trace=_trace)
    _results[0] = res
    total = np.float64(0.0)
    for r in res.results:
        total += np.float64(r["out"][0, 0])
    return np.array(np.float32(total))


# revision 10
# speedup vs baseline: 1.2245x; 1.2245x over previous
"""Trainium2 Bass kernel for nn_CustomLoss_17875653886357.

Contrastive-style loss vs. the last row (anchor) of the batch:
    lab  = (labels != labels[-1])                        [N]
    dist = ||coords - coords[-1]||^2                     [N]
    loss = sum((1-lab)*dist + lab*max(0, MARGIN-dist))   scalar

Sharding: data-parallel over N across 8 NeuronCores (4096 rows each).
The anchor row (3 floats + 1 int) is baked into the compiled kernel as
immediates; each core produces a scalar partial sum; host adds the 8
partials (the gather/unshard step).

All per-core inputs ride in ONE DMA: a [128, 226] f32 "blob" whose
partition rows pack coords (96) | anchor broadcast (96) | labels as i32
bits (32) | 1.0 | -1.0.  904B/descriptor clears the <512B DMA
latency penalty, there is a single DGE launch + semaphore set, and no
on-device memsets are needed (the matmul constants come in with the
blob) - so nothing issues before the DMA.

Raw Bacc (no Tile framework): the kernel is a straight pipeline, so
hand-placed semaphores avoid Tile's entry branches and tail
drain+barrier+sem-clear sequence.
"""

from contextlib import ExitStack

import numpy as np

import concourse.mybir as mybir
from concourse import bacc
from concourse.bass_utils import run_bass_kernel_spmd

N, D = 32768, 3
NCORES = 8
NS = N // NCORES  # rows per core = 4096
P = 128  # SBUF partitions
M = NS // P  # rows per partition = 32
MARGIN = 500.0

F32 = mybir.dt.float32
I32 = mybir.dt.int32
Alu = mybir.AluOpType

VARIANT = "dve"  # "dve" | "act"

CW = M * D  # coords block width = 96
if VARIANT == "dve":
    BW = 2 * CW + M + 2  # coords | anchor bcast | labels | 1.0 | -1.0
else:
    BW = CW + M + 4  # coords | labels | -ax | -ay | -az | 1.0


def _build(anchor_pt, anchor_lab):
    """Build the per-core Bacc program. Anchor values are compile-time
    immediates (the kernel is compiled per call, after inputs are known)."""
    if VARIANT == "act":
        return _build_act(anchor_lab)
    al = int(anchor_lab)

    nc = bacc.Bacc(
        "TRN2", target_bir_lowering=False, debug=False, enable_partition_id=False
    )
    bb = nc.cur_bb.bb
    init_names = {i.name for i in bb.instructions}
    blob_d = nc.declare_dram_parameter("blob", [P, BW], F32, isOutput=False)
    out_d = nc.declare_dram_parameter("out", [1, 1], F32, isOutput=True)

    with ExitStack() as ctx:
        BLOB = ctx.enter_context(nc.sbuf_tensor("BLOB", [P, BW], F32))
        DIFF = ctx.enter_context(nc.sbuf_tensor("DIFF", [P, CW], F32))
        SQ = ctx.enter_context(nc.sbuf_tensor("SQ", [P, CW], F32))
        E = ctx.enter_context(nc.sbuf_tensor("E", [P, M], F32))
        DN = ctx.enter_context(nc.sbuf_tensor("DN", [P, M], F32))
        H = ctx.enter_context(nc.sbuf_tensor("H", [P, M], F32))
        B = ctx.enter_context(nc.sbuf_tensor("B", [P, M], F32))
        EM = ctx.enter_context(nc.sbuf_tensor("EM", [P, M], F32))
        LOSS = ctx.enter_context(nc.sbuf_tensor("LOSS", [P, M], F32))
        RS = ctx.enter_context(nc.sbuf_tensor("RS", [P, 1], F32))
        ACC = ctx.enter_context(nc.psum_tensor("ACC", [1, 1], F32))
        in_sem = ctx.enter_context(nc.semaphore("in_sem"))
        v_sem = ctx.enter_context(nc.semaphore("v_sem"))
        g_sem = ctx.enter_context(nc.semaphore("g_sem"))
        pe_sem = ctx.enter_context(nc.semaphore("pe_sem"))
        out_sem = ctx.enter_context(nc.semaphore("out_sem"))

        ap = BLOB[:]
        C = ap[:, 0:CW]
        AB = ap[:, CW : 2 * CW]
        LI = ap[:, 2 * CW : 2 * CW + M].bitcast(I32)
        ONE = ap[:, BW - 2 : BW - 1]

        # One input DMA on sync's HW queue: all 16 phys DMA engines chew
        # the 128 blob descriptors in ~0.4us.
        dma_a = nc.sync.dma_start(BLOB[:], blob_d[:])
        dma_a.then_inc(in_sem, 16)

        # DVE instructions don't interlock with their predecessors' writes
        # (deep pipeline), so every same-engine RAW needs a semaphore hop:
        # each op bumps v_sem, dependent ops wait for the producer's count.
        vs = [0]

        def vop(inst):
            inst.then_inc(v_sem, 1)
            vs[0] += 1
            return vs[0]

        # label path on the otherwise-idle GpSimd engine: it only needs
        # the blob, so it runs concurrently with the DVE dist chain
        nc.gpsimd.wait_ge(in_sem, 16)
        nc.gpsimd.tensor_scalar(E[:], LI, al, None, Alu.is_equal).then_inc(g_sem, 1)

        # --- dist path
        nc.vector.wait_ge(in_sem, 16)
        vop(nc.vector.tensor_sub(DIFF[:], C, AB))
        nc.vector.wait_ge(v_sem, vs[0])
        vop(nc.vector.tensor_tensor(SQ[:], DIFF[:], DIFF[:], Alu.mult))
        SQ3 = SQ[:].rearrange("p (m d) -> p m d", d=D)
        nc.vector.wait_ge(v_sem, vs[0])
        vop(
            nc.vector.tensor_reduce(  # DN = -dist
                DN[:], SQ3, axis=mybir.AxisListType.X, op=Alu.add, negate=True
            )
        )
        # H = max(MARGIN - dist, 0) = max(DN + MARGIN, 0)
        nc.vector.wait_ge(v_sem, vs[0])
        h_t = vop(nc.vector.tensor_scalar(H[:], DN[:], MARGIN, 0.0, Alu.add, Alu.max))

        nc.vector.wait_ge(v_sem, h_t)
        vop(nc.vector.tensor_add(B[:], DN[:], H[:]))  # B = H - dist

        # loss = H - E*B;  RS = per-partition sum(loss)
        nc.vector.wait_ge(v_sem, vs[0])
        nc.vector.wait_ge(g_sem, 1)
        vop(nc.vector.tensor_tensor(EM[:], E[:], B[:], Alu.mult))
        nc.vector.wait_ge(v_sem, vs[0])
        rs_t = vop(
            nc.vector.scalar_tensor_tensor(
                LOSS[:], EM[:], -1.0, H[:], Alu.mult, Alu.add, accum_out=RS[:]
            )
        )

        # --- cross-partition reduction on PE: [1,1] = RS.T @ ones
        nc.tensor.wait_ge(v_sem, rs_t)
        nc.tensor.matmul(ACC[:], RS[:], ONE, start=True, stop=True).then_inc(
            pe_sem, 1
        )

        # --- result: PSUM -> SBUF (DMA cannot read PSUM), then DMA out
        OUT = ctx.enter_context(nc.sbuf_tensor("OUT", [1, 1], F32))
        nc.vector.wait_ge(pe_sem, 1)
        out_t = vop(nc.vector.tensor_copy(OUT[:], ACC[:]))
        nc.sync.wait_ge(v_sem, out_t)
        # No completion wait here: the NEFF runtime epilogue drains every
        # engine's DMA queues before signalling completion, which covers
        # this last transfer.
        nc.sync.dma_start(out_d[:], OUT[:], single_packet=True).then_inc(out_sem, 16)

    # Bass.__init__ emits per-engine const-tile memsets plus a full
    # drain + all-engine barrier. This kernel uses none of the const
    # tiles, and the NEFF runtime prologue already syncs all engines, so
    # drop them. Also hoist the HWDGE blob DMA to the very front: it
    # carries a pure access pattern (no registers), so it need not sit
    # behind the ~1.3us per-engine TPB-base loads - issuing first hides
    # that latency behind the DMA flight time.
    strip = {
        i.name
        for i in bb.instructions
        if i.name in init_names
        and type(i).__name__ in ("InstMemset", "InstDrain", "InstEventSemaphore")
    }
    front_names = {dma_a.ins.name}
    kept = [i for i in bb.instructions if i.name not in strip]
    front = [i for i in kept if i.name in front_names]
    rest = [i for i in kept if i.name not in front_names]
    idx = next(k for k, i in enumerate(rest) if i.name.endswith("dummycall")) + 1
    bb.instructions[:] = rest[:idx] + front + rest[idx:]

    nc.compile()
    return nc


def _build_act(anchor_lab):
    """ACT-offload variant: the Activation engine computes the three
    per-coordinate squares (x - a_d)^2 directly - Square with a per-
    partition bias rides in the blob - while DVE runs the label compare
    in parallel, then combines.  Shorter DVE critical chain."""
    al = int(anchor_lab)

    nc = bacc.Bacc(
        "TRN2", target_bir_lowering=False, debug=False, enable_partition_id=False
    )
    bb = nc.cur_bb.bb
    init_names = {i.name for i in bb.instructions}
    blob_d = nc.declare_dram_parameter("blob", [P, BW], F32, isOutput=False)
    out_d = nc.declare_dram_parameter("out", [1, 1], F32, isOutput=True)

    with ExitStack() as ctx:
        BLOB = ctx.enter_context(nc.sbuf_tensor("BLOB", [P, BW], F32))
        SQT = ctx.enter_context(nc.sbuf_tensor("SQT", [P, 3 * M], F32))
        E = ctx.enter_context(nc.sbuf_tensor("E", [P, M], F32))
        T01 = ctx.enter_context(nc.sbuf_tensor("T01", [P, M], F32))
        DN = ctx.enter_context(nc.sbuf_tensor("DN", [P, M], F32))
        H = ctx.enter_context(nc.sbuf_tensor("H", [P, M], F32))
        B = ctx.enter_context(nc.sbuf_tensor("B", [P, M], F32))
        EM = ctx.enter_context(nc.sbuf_tensor("EM", [P, M], F32))
        LOSS = ctx.enter_context(nc.sbuf_tensor("LOSS", [P, M], F32))
        RS = ctx.enter_context(nc.sbuf_tensor("RS", [P, 1], F32))
        ACC = ctx.enter_context(nc.psum_tensor("ACC", [1, 1], F32))
        in_sem = ctx.enter_context(nc.semaphore("in_sem"))
        a_sem = ctx.enter_context(nc.semaphore("a_sem"))
        v_sem = ctx.enter_context(nc.semaphore("v_sem"))
        pe_sem = ctx.enter_context(nc.semaphore("pe_sem"))
        out_sem = ctx.enter_context(nc.semaphore("out_sem"))

        ap = BLOB[:]
        C3 = ap[:, 0:CW].rearrange("p (m d) -> p m d", d=D)
        LI = ap[:, CW : CW + M].bitcast(I32)
        BIAS = [ap[:, CW + M + d : CW + M + d + 1] for d in range(D)]
        ONE = ap[:, BW - 1 : BW]

        dma_a = nc.sync.dma_start(BLOB[:], blob_d[:])
        dma_a.then_inc(in_sem, 16)

        vs = [0]

        def vop(inst):
            inst.then_inc(v_sem, 1)
            vs[0] += 1
            return vs[0]

        # --- squares on ACT: SQT[:, d] = (c_d - a_d)^2, no RAW between
        # the three so no same-engine hops needed
        Sq = mybir.ActivationFunctionType.Square
        nc.scalar.wait_ge(in_sem, 16)
        for d in range(D):
            nc.scalar.activation(
                SQT[:][:, d * M : (d + 1) * M], C3[:, :, d], Sq, bias=BIAS[d]
            ).then_inc(a_sem, 1)

        # --- label compare on DVE, overlapping ACT
        nc.vector.wait_ge(in_sem, 16)
        vop(nc.vector.tensor_scalar(E[:], LI, al, None, Alu.is_equal))

        # --- combine: DN = -dist = -(SQ0 + SQ1) - SQ2
        SQTa = SQT[:]
        nc.vector.wait_ge(a_sem, 2)
        vop(nc.vector.tensor_add(T01[:], SQTa[:, 0:M], SQTa[:, M : 2 * M]))
        nc.vector.wait_ge(a_sem, 3)
        nc.vector.wait_ge(v_sem, vs[0])
        vop(
            nc.vector.scalar_tensor_tensor(
                DN[:], T01[:], -1.0, SQTa[:, 2 * M : 3 * M], Alu.mult, Alu.subtract
            )
        )
        # H = max(MARGIN - dist, 0) = max(DN + MARGIN, 0)
        nc.vector.wait_ge(v_sem, vs[0])
        h_t = vop(nc.vector.tensor_scalar(H[:], DN[:], MARGIN, 0.0, Alu.add, Alu.max))

        nc.vector.wait_ge(v_sem, h_t)
        vop(nc.vector.tensor_add(B[:], DN[:], H[:]))  # B = H - dist
        nc.vector.wait_ge(v_sem, vs[0])
        vop(nc.vector.tensor_tensor(EM[:], E[:], B[:], Alu.mult))
        nc.vector.wait_ge(v_sem, vs[0])
        rs_t = vop(
            nc.vector.scalar_tensor_tensor(
                LOSS[:], EM[:], -1.0, H[:], Alu.mult, Alu.add, accum_out=RS[:]
            )
        )

        nc.tensor.wait_ge(v_sem, rs_t)
        nc.tensor.matmul(ACC[:], RS[:], ONE, start=True, stop=True).then_inc(
            pe_sem, 1
        )

        OUT = ctx.enter_context(nc.sbuf_tensor("OUT", [1, 1], F32))
        nc.vector.wait_ge(pe_sem, 1)
        out_t = vop(nc.vector.tensor_copy(OUT[:], ACC[:]))
        nc.sync.wait_ge(v_sem, out_t)
        nc.sync.dma_start(out_d[:], OUT[:], single_packet=True).then_inc(out_sem, 16)

    strip = {
        i.name
        for i in bb.instructions
        if i.name in init_names
        and type(i).__name__ in ("InstMemset", "InstDrain", "InstEventSemaphore")
    }
    front_names = {dma_a.ins.name}
    kept = [i for i in bb.instructions if i.name not in strip]
    front = [i for i in kept if i.name in front_names]
    rest = [i for i in kept if i.name not in front_names]
    idx = next(k for k, i in enumerate(rest) if i.name.endswith("dummycall")) + 1
    bb.instructions[:] = rest[:idx] + front + rest[idx:]

    nc.compile()
    return nc


_nc_cache = {}


def build_nc_and_inmaps(batched_labels, batched_predicted_coords):
    labels = np.ascontiguousarray(batched_labels)
    coords = np.ascontiguousarray(batched_predicted_coords, dtype=np.float32)
    assert labels.shape == (N,) and coords.shape == (N, D)
    if labels.dtype != np.int32:
        labels = labels.astype(np.int32)

    key = (coords[-1].tobytes(), int(labels[-1]))
    nc = _nc_cache.get(key)
    if nc is None:
        nc = _nc_cache[key] = _build(coords[-1], labels[-1])

    in_maps = []
    if VARIANT == "dve":
        ab_row = np.tile(coords[-1], M)  # [96], anchor broadcast
        for i in range(NCORES):
            sl = slice(i * NS, (i + 1) * NS)
            blob = np.empty((P, BW), np.float32)
            blob[:, 0:CW] = coords[sl].reshape(P, CW)
            blob[:, CW : 2 * CW] = ab_row
            blob[:, 2 * CW : 2 * CW + M] = (
                labels[sl].reshape(P, M).view(np.float32)
            )
            blob[:, BW - 2] = 1.0
            blob[:, BW - 1] = -1.0
            in_maps.append({"blob": blob})
    else:
        for i in range(NCORES):
            sl = slice(i * NS, (i + 1) * NS)
            blob = np.empty((P, BW), np.float32)
            blob[:, 0:CW] = coords[sl].reshape(P, CW)
            blob[:, CW : CW + M] = labels[sl].reshape(P, M).view(np.float32)
            blob[:, CW + M : CW + M + D] = -coords[-1]
            blob[:, BW - 1] = 1.0
            in_maps.append({"blob": blob})
    return nc, in_maps


def kernel(batched_labels, batched_predicted_coords, _trace=False, _results=[None]):
    nc, in_maps = build_nc_and_inmaps(batched_labels, batched_predicted_coords)
    res = run_bass_kernel_spmd(nc, in_maps, core_ids=list(range(NCORES)), trace=_trace)
    _results[0] = res
    total = np.float64(0.0)
    for r in res.results:
        total += np.float64(r["out"][0, 0])
    return np.array(np.float32(total))
